# revision 7
# baseline (speedup 1.0000x reference)
"""Trainium2 Bass kernel for LocalSensitiveAttention, data-parallel over batch on 8 cores.

Device pipeline per local batch b (layout: t on partitions in chunks of 128, e on free):
  energy = v . tanh(conv_fused(a_state) + pq + processed_memory)   via
    - conv1d(K=31)+Wl-projection+pq folded into one K=32 matmul per t-chunk
      (lhsT = 31 shifted copies of a_state + ones row; rhs = [W2^T; pq_b])
    - processed_memory added by an identity matmul accumulating into the same PSUM
    - tanh on ACT straight from PSUM; (tanh*v -> reduce) fused in one DVE ttr per chunk
  softmax without max-subtraction (|energy| <= ~10 so fp32 exp is safe), sum via
    ACT exp accum_out + all-ones matmul partition-broadcast, DVE reciprocal
  context = sum_t a_t * mem[t,:] as 16 accumulating [128,1]x[128,512] matmuls
"""

import os
import sys
from contextlib import ExitStack

import numpy as np

for _p in ("/opt/trn_rl_repo", "/root/.axon_site/_ro/trn_rl_repo"):
    if os.path.isdir(_p) and _p not in sys.path:
        sys.path.insert(0, _p)

import concourse.bass as bass  # noqa: E402
import concourse.bacc as bacc  # noqa: E402
import concourse.tile as tile  # noqa: E402
from concourse import mybir  # noqa: E402
from concourse.bass_utils import run_bass_kernel_spmd  # noqa: E402

B, T = 64, 2048
E, Q, M, KC, F = 128, 1024, 512, 31, 32
NCORES = 8
BL = B // NCORES  # local batches per core
NT = T // 128  # 16 t-chunks
f32 = mybir.dt.float32

_STATE = {}


def _emit(nc):
    Tanh = mybir.ActivationFunctionType.Tanh
    Exp = mybir.ActivationFunctionType.Exp
    mult = mybir.AluOpType.mult
    add = mybir.AluOpType.add

    shifted_d = nc.dram_tensor("shifted", [BL * 32, T], f32, kind="ExternalInput")
    wpq_d = nc.dram_tensor("wpq", [BL * 32, E], f32, kind="ExternalInput")
    pm_d = nc.dram_tensor("pm", [BL * T, E], f32, kind="ExternalInput")
    mem_d = nc.dram_tensor("mem", [BL * T, M], f32, kind="ExternalInput")
    ast_d = nc.dram_tensor("astate", [BL, T], f32, kind="ExternalInput")
    vb_d = nc.dram_tensor("vb", [128, 512], f32, kind="ExternalInput")
    id_d = nc.dram_tensor("id128", [128, 128], f32, kind="ExternalInput")
    ones_d = nc.dram_tensor("ones128", [128, 128], f32, kind="ExternalInput")
    ctx_d = nc.dram_tensor("ctx_out", [BL, M], f32, kind="ExternalOutput")
    al_d = nc.dram_tensor("align_out", [BL, T], f32, kind="ExternalOutput")
    ns_d = nc.dram_tensor("next_out", [BL, T], f32, kind="ExternalOutput")

    with tile.TileContext(nc) as tc, ExitStack() as ctx:
        const = ctx.enter_context(tc.tile_pool(name="const", bufs=1))
        p_mem = ctx.enter_context(tc.tile_pool(name="mem", bufs=3))
        p_pm = ctx.enter_context(tc.tile_pool(name="pm", bufs=3))
        p_sh = ctx.enter_context(tc.tile_pool(name="sh", bufs=2))
        p_th = ctx.enter_context(tc.tile_pool(name="th", bufs=3))
        p_sm = ctx.enter_context(tc.tile_pool(name="sm", bufs=2))
        ps_conv = ctx.enter_context(tc.tile_pool(name="psc", bufs=2, space="PSUM"))
        ps_ctx = ctx.enter_context(tc.tile_pool(name="psx", bufs=2, space="PSUM"))
        ps_sbc = ctx.enter_context(tc.tile_pool(name="psb", bufs=2, space="PSUM"))
        ps_alt = ctx.enter_context(tc.tile_pool(name="psa", bufs=2, space="PSUM"))

        vb = const.tile([128, 512], f32)
        nc.sync.dma_start(vb[:], vb_d.ap())
        idm = const.tile([128, 128], f32)
        nc.sync.dma_start(idm[:], id_d.ap())
        onesm = const.tile([128, 128], f32)
        nc.sync.dma_start(onesm[:], ones_d.ap())

        for b in range(BL):
            sh = p_sh.tile([32, T], f32)
            nc.sync.dma_start(sh[:], shifted_d.ap()[b * 32 : (b + 1) * 32, :])
            Rb = p_sh.tile([32, E], f32, tag="Rb")
            nc.sync.dma_start(Rb[:], wpq_d.ap()[b * 32 : (b + 1) * 32, :])
            pmb = p_pm.tile([128, NT * E], f32)
            nc.sync.dma_start(
                pmb[:].rearrange("p (c e) -> p c e", c=NT),
                pm_d.ap()[b * T : (b + 1) * T, :].rearrange("(c p) e -> p c e", p=128),
            )
            memb = p_mem.tile([128, NT * M], f32)
            nc.sync.dma_start(
                memb[:].rearrange("p (j m) -> p j m", j=NT),
                mem_d.ap()[b * T : (b + 1) * T, :].rearrange("(j p) m -> p j m", p=128),
            )
            asb = p_sm.tile([16, 128], f32, tag="asb")
            nc.sync.dma_start(
                asb[:], ast_d.ap()[b : b + 1, :].rearrange("o (c p) -> (o c) p", p=128)
            )

            energy = p_sm.tile([128, NT], f32, tag="energy")
            for g in range(4):
                psg = ps_conv.tile([128, 512], f32)
                for ci in range(4):
                    c = g * 4 + ci
                    nc.tensor.matmul(
                        psg[:, ci * 128 : (ci + 1) * 128],
                        sh[:, c * 128 : (c + 1) * 128],
                        Rb[:],
                        start=(ci == 0),
                        stop=False,
                    )
                # += processed_memory for these 4 chunks via identity matmul
                nc.tensor.matmul(
                    psg[:],
                    idm[:],
                    pmb[:, g * 512 : (g + 1) * 512],
                    start=False,
                    stop=True,
                )
                th = p_th.tile([128, 512], f32)
                nc.scalar.activation(th[:], psg[:], Tanh)
                scr = p_th.tile([128, 512], f32, tag="scr")
                nc.vector.tensor_mul(scr[:], th[:], vb[:])
                nc.vector.tensor_reduce(
                    energy[:, g * 4 : (g + 1) * 4],
                    scr[:].rearrange("p (c e) -> p c e", c=4),
                    axis=mybir.AxisListType.X,
                    op=add,
                )

            # softmax over all T (128 partitions x 16 cols), no max subtraction
            expt = p_sm.tile([128, NT], f32, tag="expt")
            esum = p_sm.tile([128, 1], f32, tag="esum")
            nc.scalar.activation(expt[:], energy[:], Exp, accum_out=esum[:])
            psb = ps_sbc.tile([128, 1], f32)
            nc.tensor.matmul(psb[:], onesm[:], esum[:], start=True, stop=True)
            rec = p_sm.tile([128, 1], f32, tag="rec")
            nc.vector.reciprocal(rec[:], psb[:])
            alg = p_sm.tile([128, NT], f32, tag="alg")
            nc.vector.tensor_scalar_mul(alg[:], expt[:], rec[:])

            # alignments to [16,128] layout for output + next_state
            pal = ps_alt.tile([16, 128], f32)
            nc.tensor.transpose(pal[:], alg[:], idm[:])
            alT = p_sm.tile([16, 128], f32, tag="alT")
            nc.vector.tensor_copy(alT[:], pal[:])
            nxt = p_sm.tile([16, 128], f32, tag="nxt")
            nc.vector.tensor_add(nxt[:], pal[:], asb[:])

            # context = sum_t a_t * mem[t, :]
            psx = ps_ctx.tile([1, 512], f32)
            for j in range(NT):
                nc.tensor.matmul(
                    psx[:],
                    alg[:, j : j + 1],
                    memb[:, j * 512 : (j + 1) * 512],
                    start=(j == 0),
                    stop=(j == NT - 1),
                )
            ctxs = p_sm.tile([1, 512], f32, tag="ctxs")
            nc.scalar.copy(ctxs[:], psx[:])

            nc.sync.dma_start(
                al_d.ap()[b : b + 1, :].rearrange("o (c p) -> (o c) p", p=128), alT[:]
            )
            nc.sync.dma_start(
                ns_d.ap()[b : b + 1, :].rearrange("o (c p) -> (o c) p", p=128), nxt[:]
            )
            nc.sync.dma_start(ctx_d.ap()[b : b + 1, :], ctxs[:])


def _get_nc():
    if "nc" not in _STATE:
        nc = bacc.Bacc("TRN2", target_bir_lowering=False, debug=False, num_devices=NCORES)
        _emit(nc)
        nc.compile()
        _STATE["nc"] = nc
    return _STATE["nc"]


def _prep_in_maps(query, alignment_state, memory, processed_memory, Wq, bq, conv_w, Wl, v):
    fq = np.float32
    query = np.asarray(query, fq)
    alignment_state = np.asarray(alignment_state, fq)
    memory = np.asarray(memory, fq)
    processed_memory = np.asarray(processed_memory, fq)
    Wq = np.asarray(Wq, fq)
    bq = np.asarray(bq, fq)
    conv_w = np.asarray(conv_w, fq)
    Wl = np.asarray(Wl, fq)
    v = np.asarray(v, fq)

    pq = query @ Wq.T + bq  # [B, E]
    W2T = (Wl @ conv_w[:, 0, :]).T  # [31, E] ; W2T[k, e] = sum_f Wl[e,f] w[f,k]

    padded = np.zeros((B, T + KC - 1), fq)
    padded[:, (KC - 1) // 2 : (KC - 1) // 2 + T] = alignment_state
    from numpy.lib.stride_tricks import sliding_window_view

    win = sliding_window_view(padded, T, axis=1)  # [B, 31, T]; win[b,k,t] = a[b, t+k-15]
    shifted = np.empty((B, 32, T), fq)
    shifted[:, :KC] = win
    shifted[:, KC] = 1.0

    wpq = np.empty((B, 32, E), fq)
    wpq[:, :KC] = W2T[None]
    wpq[:, KC] = pq

    vbt = np.ascontiguousarray(np.tile(v, (128, 4)))
    id128 = np.eye(128, dtype=fq)
    ones128 = np.ones((128, 128), fq)

    in_maps = []
    for i in range(NCORES):
        sl = slice(i * BL, (i + 1) * BL)
        in_maps.append(
            dict(
                shifted=np.ascontiguousarray(shifted[sl].reshape(BL * 32, T)),
                wpq=np.ascontiguousarray(wpq[sl].reshape(BL * 32, E)),
                pm=np.ascontiguousarray(processed_memory[sl].reshape(BL * T, E)),
                mem=np.ascontiguousarray(memory[sl].reshape(BL * T, M)),
                astate=np.ascontiguousarray(alignment_state[sl]),
                vb=vbt,
                id128=id128,
                ones128=ones128,
            )
        )
    return in_maps


def run_sharded(inputs, trace=False):
    nc = _get_nc()
    in_maps = _prep_in_maps(
        inputs["query"],
        inputs["alignment_state"],
        inputs["memory"],
        inputs["processed_memory"],
        inputs["Wq"],
        inputs["bq"],
        inputs["conv_w"],
        inputs["Wl"],
        inputs["v"],
    )
    res = run_bass_kernel_spmd(nc, in_maps, list(range(NCORES)), trace=trace)
    outs = res.results
    ctx = np.concatenate([np.asarray(r["ctx_out"]) for r in outs], axis=0)
    align = np.concatenate([np.asarray(r["align_out"]) for r in outs], axis=0)
    nxt = np.concatenate([np.asarray(r["next_out"]) for r in outs], axis=0)
    return (ctx, align, nxt), res


def kernel(query, alignment_state, memory, processed_memory, mask=None, Wq=None, bq=None, conv_w=None, Wl=None, v=None):
    (ctx, align, nxt), _ = run_sharded(
        dict(
            query=query,
            alignment_state=alignment_state,
            memory=memory,
            processed_memory=processed_memory,
            Wq=Wq,
            bq=bq,
            conv_w=conv_w,
            Wl=Wl,
            v=v,
        )
    )
    return ctx, align, nxt


# revision 9
# speedup vs baseline: 718.5026x; 718.5026x over previous
"""Trainium2 Bass kernel for LocalSensitiveAttention, data-parallel over batch on 8 cores.

Device pipeline per local batch b (layout: t on partitions in chunks of 128, e on free):
  energy = v . tanh(conv_fused(a_state) + pq + processed_memory)   via
    - conv1d(K=31)+Wl-projection+pq folded into one K=32 matmul per t-chunk
      (lhsT = 31 shifted copies of a_state + ones row; rhs = [W2^T; pq_b])
    - processed_memory added by an identity matmul accumulating into the same PSUM
    - tanh on ACT straight from PSUM; (tanh*v -> reduce) fused in one DVE ttr per chunk
  softmax without max-subtraction (|energy| <= ~10 so fp32 exp is safe), sum via
    ACT exp accum_out + all-ones matmul partition-broadcast, DVE reciprocal
  context = sum_t a_t * mem[t,:] as 16 accumulating [128,1]x[128,512] matmuls
"""

import os
import sys
from contextlib import ExitStack

import numpy as np

for _p in ("/opt/trn_rl_repo", "/root/.axon_site/_ro/trn_rl_repo"):
    if os.path.isdir(_p) and _p not in sys.path:
        sys.path.insert(0, _p)

import concourse.bass as bass  # noqa: E402
import concourse.bacc as bacc  # noqa: E402
import concourse.tile as tile  # noqa: E402
from concourse import mybir  # noqa: E402
from concourse.bass_utils import run_bass_kernel_spmd  # noqa: E402

B, T = 64, 2048
E, Q, M, KC, F = 128, 1024, 512, 31, 32
NCORES = 8
BL = B // NCORES  # local batches per core
NT = T // 128  # 16 t-chunks
f32 = mybir.dt.float32

_STATE = {}


def _emit(nc):
    Tanh = mybir.ActivationFunctionType.Tanh
    Exp = mybir.ActivationFunctionType.Exp
    mult = mybir.AluOpType.mult
    add = mybir.AluOpType.add

    shifted_d = nc.dram_tensor("shifted", [BL * 32, T], f32, kind="ExternalInput")
    wpq_d = nc.dram_tensor("wpq", [BL * 32, E], f32, kind="ExternalInput")
    pm_d = nc.dram_tensor("pm", [BL * T, E], f32, kind="ExternalInput")
    mem_d = nc.dram_tensor("mem", [BL * T, M], f32, kind="ExternalInput")
    vb_d = nc.dram_tensor("vb", [128, 512], f32, kind="ExternalInput")
    id_d = nc.dram_tensor("id128", [128, 128], f32, kind="ExternalInput")
    ones_d = nc.dram_tensor("ones128", [128, 128], f32, kind="ExternalInput")
    ctx_d = nc.dram_tensor("ctx_out", [BL, M], f32, kind="ExternalOutput")
    al_d = nc.dram_tensor("align_out", [BL, T], f32, kind="ExternalOutput")

    with tile.TileContext(nc) as tc, ExitStack() as ctx:
        const = ctx.enter_context(tc.tile_pool(name="const", bufs=1))
        p_mem = ctx.enter_context(tc.tile_pool(name="mem", bufs=3))
        p_pm = ctx.enter_context(tc.tile_pool(name="pm", bufs=3))
        p_sh = ctx.enter_context(tc.tile_pool(name="sh", bufs=2))
        p_th = ctx.enter_context(tc.tile_pool(name="th", bufs=3))
        p_sm = ctx.enter_context(tc.tile_pool(name="sm", bufs=2))
        ps_conv = ctx.enter_context(tc.tile_pool(name="psc", bufs=2, space="PSUM"))
        ps_ctx = ctx.enter_context(tc.tile_pool(name="psx", bufs=2, space="PSUM"))
        ps_sbc = ctx.enter_context(tc.tile_pool(name="psb", bufs=2, space="PSUM"))
        ps_alt = ctx.enter_context(tc.tile_pool(name="psa", bufs=2, space="PSUM"))

        vb = const.tile([128, 512], f32)
        nc.sync.dma_start(vb[:], vb_d.ap())
        idm = const.tile([128, 128], f32)
        nc.sync.dma_start(idm[:], id_d.ap())
        onesm = const.tile([128, 128], f32)
        nc.sync.dma_start(onesm[:], ones_d.ap())

        for b in range(BL):
            sh = p_sh.tile([32, T], f32)
            nc.sync.dma_start(sh[:], shifted_d.ap()[b * 32 : (b + 1) * 32, :])
            Rb = p_sh.tile([32, E], f32, tag="Rb")
            nc.sync.dma_start(Rb[:], wpq_d.ap()[b * 32 : (b + 1) * 32, :])
            pmb = p_pm.tile([128, NT * E], f32)
            nc.sync.dma_start(
                pmb[:].rearrange("p (c e) -> p c e", c=NT),
                pm_d.ap()[b * T : (b + 1) * T, :].rearrange("(p c) e -> p c e", p=128),
            )
            memb = p_mem.tile([128, NT * M], f32)
            nc.sync.dma_start(
                memb[:].rearrange("p (j m) -> p j m", j=NT),
                mem_d.ap()[b * T : (b + 1) * T, :].rearrange("(p j) m -> p j m", p=128),
            )
            energy = p_sm.tile([128, NT], f32, tag="energy")
            for g in range(4):
                psg = ps_conv.tile([128, 512], f32)
                for ci in range(4):
                    c = g * 4 + ci
                    nc.tensor.matmul(
                        psg[:, ci * 128 : (ci + 1) * 128],
                        sh[:, c * 128 : (c + 1) * 128],
                        Rb[:],
                        start=(ci == 0),
                        stop=(ci == 3),
                    )
                xg = p_th.tile([128, 512], f32, tag="xg")
                nc.vector.tensor_add(xg[:], psg[:], pmb[:, g * 512 : (g + 1) * 512])
                th = p_th.tile([128, 512], f32)
                nc.scalar.activation(th[:], xg[:], Tanh)
                scr = p_th.tile([128, 512], f32, tag="scr")
                nc.vector.tensor_mul(scr[:], th[:], vb[:])
                nc.vector.tensor_reduce(
                    energy[:, g * 4 : (g + 1) * 4],
                    scr[:].rearrange("p (c e) -> p c e", c=4),
                    axis=mybir.AxisListType.X,
                    op=add,
                )

            # softmax over all T (128 partitions x 16 cols), no max subtraction
            expt = p_sm.tile([128, NT], f32, tag="expt")
            esum = p_sm.tile([128, 1], f32, tag="esum")
            nc.scalar.activation(expt[:], energy[:], Exp, accum_out=esum[:])
            psb = ps_sbc.tile([128, 1], f32)
            nc.tensor.matmul(psb[:], onesm[:], esum[:], start=True, stop=True)
            rec = p_sm.tile([128, 1], f32, tag="rec")
            nc.vector.reciprocal(rec[:], psb[:])
            alg = p_sm.tile([128, NT], f32, tag="alg")
            nc.vector.tensor_scalar_mul(alg[:], expt[:], rec[:])

            # alignments to [16,128] layout for output + next_state
            pal = ps_alt.tile([16, 128], f32)
            nc.tensor.transpose(pal[:], alg[:], idm[:])
            alT = p_sm.tile([16, 128], f32, tag="alT")
            nc.vector.tensor_copy(alT[:], pal[:])

            # context = sum_t a_t * mem[t, :]
            psx = ps_ctx.tile([1, 512], f32)
            for j in range(NT):
                nc.tensor.matmul(
                    psx[:],
                    alg[:, j : j + 1],
                    memb[:, j * 512 : (j + 1) * 512],
                    start=(j == 0),
                    stop=(j == NT - 1),
                )
            ctxs = p_sm.tile([1, 512], f32, tag="ctxs")
            nc.scalar.copy(ctxs[:], psx[:])

            nc.sync.dma_start(
                al_d.ap()[b : b + 1, :].rearrange("o (j p) -> (o j) p", p=128), alT[:]
            )
            nc.sync.dma_start(ctx_d.ap()[b : b + 1, :], ctxs[:])


def _get_nc():
    if "nc" not in _STATE:
        nc = bacc.Bacc("TRN2", target_bir_lowering=False, debug=False, num_devices=NCORES)
        _emit(nc)
        nc.compile()
        _STATE["nc"] = nc
    return _STATE["nc"]


def _prep_in_maps(query, alignment_state, memory, processed_memory, Wq, bq, conv_w, Wl, v):
    fq = np.float32
    query = np.asarray(query, fq)
    alignment_state = np.asarray(alignment_state, fq)
    memory = np.asarray(memory, fq)
    processed_memory = np.asarray(processed_memory, fq)
    Wq = np.asarray(Wq, fq)
    bq = np.asarray(bq, fq)
    conv_w = np.asarray(conv_w, fq)
    Wl = np.asarray(Wl, fq)
    v = np.asarray(v, fq)

    pq = query @ Wq.T + bq  # [B, E]
    W2T = (Wl @ conv_w[:, 0, :]).T  # [31, E] ; W2T[k, e] = sum_f Wl[e,f] w[f,k]

    padded = np.zeros((B, T + KC - 1), fq)
    padded[:, (KC - 1) // 2 : (KC - 1) // 2 + T] = alignment_state
    from numpy.lib.stride_tricks import sliding_window_view

    win = sliding_window_view(padded, T, axis=1)  # [B, 31, T]; win[b,k,t] = a[b, t+k-15]
    shifted = np.empty((B, 32, T), fq)
    shifted[:, :KC] = win
    shifted[:, KC] = 1.0
    # p-major permute: column j*128+p holds t = p*16 + j
    shifted = np.ascontiguousarray(
        shifted.reshape(B, 32, 128, 16).swapaxes(2, 3).reshape(B, 32, T)
    )

    wpq = np.empty((B, 32, E), fq)
    wpq[:, :KC] = W2T[None]
    wpq[:, KC] = pq

    vbt = np.ascontiguousarray(np.tile(v, (128, 4)))
    id128 = np.eye(128, dtype=fq)
    ones128 = np.ones((128, 128), fq)

    in_maps = []
    for i in range(NCORES):
        sl = slice(i * BL, (i + 1) * BL)
        in_maps.append(
            dict(
                shifted=np.ascontiguousarray(shifted[sl].reshape(BL * 32, T)),
                wpq=np.ascontiguousarray(wpq[sl].reshape(BL * 32, E)),
                pm=np.ascontiguousarray(processed_memory[sl].reshape(BL * T, E)),
                mem=np.ascontiguousarray(memory[sl].reshape(BL * T, M)),
                vb=vbt,
                id128=id128,
                ones128=ones128,
            )
        )
    return in_maps


def run_sharded(inputs, trace=False):
    nc = _get_nc()
    in_maps = _prep_in_maps(
        inputs["query"],
        inputs["alignment_state"],
        inputs["memory"],
        inputs["processed_memory"],
        inputs["Wq"],
        inputs["bq"],
        inputs["conv_w"],
        inputs["Wl"],
        inputs["v"],
    )
    res = run_bass_kernel_spmd(nc, in_maps, list(range(NCORES)), trace=trace)
    outs = res.results
    ctx = np.concatenate([np.asarray(r["ctx_out"]) for r in outs], axis=0)
    align_pm = np.concatenate([np.asarray(r["align_out"]) for r in outs], axis=0)
    # device wrote j-major: flat[j*128 + p] = a[p*16 + j]; unpermute
    align = np.ascontiguousarray(
        align_pm.reshape(B, 16, 128).swapaxes(1, 2).reshape(B, T)
    )
    nxt = np.asarray(inputs["alignment_state"], np.float32) + align
    return (ctx, align, nxt), res


def kernel(query, alignment_state, memory, processed_memory, mask=None, Wq=None, bq=None, conv_w=None, Wl=None, v=None):
    (ctx, align, nxt), _ = run_sharded(
        dict(
            query=query,
            alignment_state=alignment_state,
            memory=memory,
            processed_memory=processed_memory,
            Wq=Wq,
            bq=bq,
            conv_w=conv_w,
            Wl=Wl,
            v=v,
        )
    )
    return ctx, align, nxt
